# revision 4
# baseline (speedup 1.0000x reference)
"""Trainium2 Bass kernel for nn_AttentionBlockSE3 (SE3 graph attention block).

Reference computation (N=20000 nodes, E=320000 edges, C=64 channels, H=8 heads):
  k = to_heads(key_0, key_1)      [E, 8, 32]
  q = to_heads(query_0, query_1)  [N, 8, 32]
  logits = einsum('ehd,ehd->eh', k, q[dst]) / 16
  alpha  = edge_softmax(logits, dst)           (per dst node, per head)
  out_d  = segment_sum(alpha * value_d, dst)   for degree 0 and 1 values
  returns (out_0 [N,64,1], out_1 [N,64,3], prelogits [E,8])

Strategy (edge-parallel across 8 NeuronCores, no collectives needed):
  * Host sorts edges by dst and groups nodes into tiles of 127 (+1 trash slot
    that absorbs padding edges).  Each tile's edges are padded to whole
    128-edge chunks.  Tiles are dealt to the 8 cores so that every core has an
    IDENTICAL compile-time schedule cnt[slot] (SPMD: one program, 8 cores).
  * Per chunk the device builds a one-hot edge->node-slot matrix from the dst
    values (iota + tensor_scalar eq), gathers q via a PE matmul, computes the
    per-head logits on DVE, exp on ACT (softmax without max subtraction -- the
    logits are O(1) so this is numerically safe and matches the reference
    up to fp32 rounding), and scatter-adds the exp-weighted values and the
    softmax denominators into a per-tile PSUM accumulator via a second PE
    matmul.  At the tile boundary the accumulator is divided by the
    denominators and written out.
  * Host inverse-permutes the outputs.
"""

import math
import numpy as np

from concourse import bass, mybir
from concourse.tile import TileContext, ScopedClock
from concourse.bass_utils import run_bass_kernel_spmd

# ----------------------------------------------------------------------------
# Workaround: this walrus build accepts at most ONE sem wait per instruction
# ("Too many sync wait commands").  Split extra waits onto single-wait Drain
# carriers on the same engine, and split the kernel-tail drain the same way.
# ----------------------------------------------------------------------------
_PATCHED = False


def _patch_tile():
    global _PATCHED
    if _PATCHED:
        return
    _PATCHED = True

    orig_add = TileContext._add_instruction

    def _add_instruction(self, inst):
        si = getattr(inst, "sync_info", None)
        if si is not None and si.on_wait and len(si.on_wait) > 1:
            waits = list(si.on_wait)
            for w in waits[:-1]:
                nop = mybir.InstDrain(
                    name=self.nc.get_next_instruction_name(), ins=[], outs=[]
                )
                nop.engine = inst.engine
                nop.sync_info = mybir.SyncInfo(on_wait=[w], on_update=[])
                orig_add(self, nop)
            while len(si.on_wait) > 1:
                si.on_wait.pop(0)
            inst.sync_info = si
        orig_add(self, inst)

    TileContext._add_instruction = _add_instruction

    def _drain_and_barrier(self, tick_clock, wait_clock):
        drain_inst = self.nc.sync.drain()
        wait_clock.add_sem_waits(
            drain_inst.ins, ScopedClock({None: tick_clock.global_clock})
        )
        si = drain_inst.ins.sync_info
        waits = list(si.on_wait) if si and si.on_wait else []
        if len(waits) > 1:
            while len(si.on_wait) > 1:
                si.on_wait.pop()
            drain_inst.ins.sync_info = si
            for w in waits[1:]:
                extra = self.nc.sync.drain()
                esi = extra.ins.sync_info
                if esi is None:
                    esi = mybir.SyncInfo(on_wait=[w], on_update=[])
                else:
                    esi.on_wait.append(w)
                extra.ins.sync_info = esi
        self.nc.all_engine_barrier()
        assert self.sems is not None
        popped = self.nc._tile_sem_poison_stack.pop()
        assert popped is self._sem_poison
        self.nc.clear_and_free_semaphores(list(self.sems.allocated().values()))
        self.nc.all_engine_barrier()

    TileContext._drain_and_barrier = _drain_and_barrier


# ----------------------------------------------------------------------------
# Problem constants
# ----------------------------------------------------------------------------
P = 128          # partitions / edges per chunk / node slots per tile
NPT = 127        # real nodes per tile (slot 127 = trash)
NCORES = 8
H = 8            # heads
FK = 256         # head-major k/q features (8 heads x (8 + 24))
HB = 33          # per-head block in wvw/acc: 8 (v0) + 24 (v1) + 1 (w)
W = H * HB       # 264


# ----------------------------------------------------------------------------
# Host-side preprocessing
# ----------------------------------------------------------------------------
def _plan(dst, N):
    """Sort edges by dst, tile nodes, deal tiles to cores with a shared
    per-slot chunk-count schedule.  Returns the schedule + index arrays."""
    E = dst.shape[0]
    dst = dst.astype(np.int64, copy=False)
    perm = np.argsort(dst, kind="stable")
    dst_s = dst[perm]

    n_tiles = (N + NPT - 1) // NPT
    bound = np.minimum(np.arange(n_tiles + 1) * NPT, N)
    starts = np.searchsorted(dst_s, bound[:-1])
    ends = np.searchsorted(dst_s, bound[1:])
    counts = ends - starts                      # edges per tile
    chunks_t = (counts + P - 1) // P            # chunks per tile

    T = (n_tiles + NCORES - 1) // NCORES        # tile slots per core
    order = np.argsort(-chunks_t, kind="stable")

    core_tiles = np.full((NCORES, T), -1, dtype=np.int64)
    cnt = np.zeros(T, dtype=np.int64)
    for s in range(T):
        grp = order[s * NCORES:(s + 1) * NCORES]
        core_tiles[: len(grp), s] = grp
        cnt[s] = chunks_t[grp].max() if len(grp) else 0

    keep = cnt > 0
    cnt = cnt[keep]
    core_tiles = core_tiles[:, keep]
    T = int(cnt.shape[0])
    n_chunks = int(cnt.sum())
    chunk_off = np.concatenate([[0], np.cumsum(cnt)])[:-1]  # per slot

    return dict(perm=perm, dst_s=dst_s, starts=starts, counts=counts,
                core_tiles=core_tiles, cnt=cnt, chunk_off=chunk_off,
                T=T, n_chunks=n_chunks, E=E, N=N)


def _build_core_inputs(plan, key_0, key_1, query_0, query_1, value_0, value_1):
    """Per-core kv / dstloc / q arrays + row->original-edge index maps."""
    E, N = plan["E"], plan["N"]
    T, n_chunks = plan["T"], plan["n_chunks"]
    cnt, chunk_off = plan["cnt"], plan["chunk_off"]
    core_tiles = plan["core_tiles"]
    perm, dst_s, starts, counts = (plan["perm"], plan["dst_s"],
                                   plan["starts"], plan["counts"])

    k0 = key_0.reshape(E, H, 8)
    k1 = key_1.reshape(E, H, 24)
    v0 = value_0.reshape(E, H, 8)
    v1 = value_1.reshape(E, H, 24)
    qhm = np.concatenate(
        [query_0.reshape(N, H, 8), query_1.reshape(N, H, 24)], axis=2
    ).reshape(N, FK).astype(np.float32, copy=False)

    rows = n_chunks * P
    ins, metas = [], []
    for c in range(NCORES):
        orig = np.full(rows, -1, dtype=np.int64)
        dl = np.full(rows, NPT, dtype=np.float32)       # pads -> trash slot
        qt = np.zeros((T * P, FK), dtype=np.float32)
        for s in range(T):
            tid = core_tiles[c, s]
            if tid < 0:
                continue
            lo = tid * NPT
            hi = min(lo + NPT, N)
            qt[s * P: s * P + (hi - lo)] = qhm[lo:hi]
            st, ce = starts[tid], counts[tid]
            r0 = chunk_off[s] * P
            orig[r0: r0 + ce] = perm[st: st + ce]
            dl[r0: r0 + ce] = (dst_s[st: st + ce] - lo).astype(np.float32)

        valid = orig >= 0
        oi = orig[valid]
        kv = np.zeros((rows, 512), dtype=np.float32)
        kvk = kv[:, 0:256].reshape(rows, H, 32)
        kvv = kv[:, 256:512].reshape(rows, H, 32)
        kvk[valid, :, 0:8] = k0[oi]
        kvk[valid, :, 8:32] = k1[oi]
        kvv[valid, :, 0:8] = v0[oi]
        kvv[valid, :, 8:32] = v1[oi]

        dstloc = np.ascontiguousarray(dl.reshape(n_chunks, P).T)  # [P, n_chunks]
        ins.append({
            "kv": kv.reshape(n_chunks, P, 512),
            "dl": dstloc,
            "qt": qt.reshape(T, P, FK),
            "iota": np.broadcast_to(
                np.arange(P, dtype=np.float32), (P, P)).copy(),
            "ident": np.eye(P, dtype=np.float32),
        })
        metas.append(dict(orig=orig, valid=valid))
    return ins, metas


# ----------------------------------------------------------------------------
# Device program
# ----------------------------------------------------------------------------
def _build_program(T, n_chunks, cnt, use_f32r=False):
    _patch_tile()
    nc = bass.Bass("TRN2", target_bir_lowering=False, debug=False,
                   num_devices=NCORES)
    f32 = mybir.dt.float32
    f32r = mybir.dt.float32r
    mm_dt = f32r if use_f32r else f32

    kv = nc.dram_tensor("kv", [n_chunks, P, 512], f32, kind="ExternalInput").ap()
    dl = nc.dram_tensor("dl", [P, n_chunks], f32, kind="ExternalInput").ap()
    qt = nc.dram_tensor("qt", [T, P, FK],
                        mm_dt, kind="ExternalInput").ap()
    iota = nc.dram_tensor("iota", [P, P], f32, kind="ExternalInput").ap()
    ident = nc.dram_tensor("ident", [P, P], f32, kind="ExternalInput").ap()

    outd = nc.dram_tensor("out", [T, P, 256], f32, kind="ExternalOutput").ap()
    pld = nc.dram_tensor("pl", [n_chunks, P, H], f32, kind="ExternalOutput").ap()

    cnt = [int(x) for x in cnt]
    with TileContext(nc) as tc:
        with (
            tc.tile_pool(name="const", bufs=1) as constp,
            tc.tile_pool(name="qp", bufs=2) as qp,
            tc.tile_pool(name="kvp", bufs=4) as kvp,
            tc.tile_pool(name="ohp", bufs=3) as ohp,
            tc.tile_pool(name="ohnep", bufs=3) as ohnep,
            tc.tile_pool(name="prodp", bufs=2) as prodp,
            tc.tile_pool(name="lgp", bufs=3) as lgp,
            tc.tile_pool(name="wvwp", bufs=3) as wvwp,
            tc.tile_pool(name="plp", bufs=2) as plp,
            tc.tile_pool(name="fin", bufs=2) as finp,
            tc.tile_pool(name="psoh", bufs=2, space="PSUM") as psoh,
            tc.tile_pool(name="psqg", bufs=2, space="PSUM") as psqg,
            tc.tile_pool(name="psacc", bufs=2, space="PSUM") as psacc,
        ):
            iota_t = constp.tile([P, P], f32)
            nc.sync.dma_start(out=iota_t[:, :], in_=iota[:, :])
            ident_t = constp.tile([P, P], f32)
            nc.sync.dma_start(out=ident_t[:, :], in_=ident[:, :])
            dl_t = constp.tile([P, n_chunks], f32)
            nc.sync.dma_start(out=dl_t[:, :], in_=dl[:, :])

            ch = 0
            for s in range(T):
                cs = cnt[s]
                q_t = qp.tile([P, FK], mm_dt, tag="q")
                nc.sync.dma_start(out=q_t[:, :], in_=qt[s])
                acc = psacc.tile([P, W], f32, tag="acc")
                acc_v = acc[:, :].rearrange("p (h c) -> p h c", h=H)
                pl_t = plp.tile([P, cs * H], f32, tag="pl")

                for c in range(cs):
                    kvt = kvp.tile([P, 512], f32, tag="kv")
                    nc.sync.dma_start(out=kvt[:, :], in_=kv[ch])

                    oh_en = ohp.tile([P, P], mm_dt, tag="oh")
                    nc.vector.tensor_scalar(
                        out=oh_en[:, :], in0=iota_t[:, :],
                        scalar1=dl_t[:, ch:ch + 1], scalar2=None,
                        op0=mybir.AluOpType.is_equal)

                    oh_ps = psoh.tile([P, P], f32, tag="ohps")
                    nc.tensor.transpose(oh_ps[:, :], oh_en[:, :].bitcast(f32),
                                        ident_t[:, :])
                    oh_ne = ohnep.tile([P, P], mm_dt, tag="ohne")
                    nc.scalar.copy(out=oh_ne[:, :], in_=oh_ps[:, :])

                    qg = psqg.tile([P, FK], f32, tag="qg")
                    nc.tensor.matmul(qg[:, :], oh_ne[:, :], q_t[:, :],
                                     start=True, stop=True)

                    prod = prodp.tile([P, FK], f32, tag="prod")
                    nc.vector.tensor_tensor(
                        out=prod[:, :], in0=kvt[:, 0:FK], in1=qg[:, :],
                        op=mybir.AluOpType.mult)
                    logits = lgp.tile([P, H], f32, tag="lg")
                    nc.vector.tensor_reduce(
                        out=logits[:, :],
                        in_=prod[:, :].rearrange("p (h d) -> p h d", h=H),
                        axis=mybir.AxisListType.X, op=mybir.AluOpType.add)

                    nc.scalar.mul(out=pl_t[:, c * H:(c + 1) * H],
                                  in_=logits[:, :], mul=1.0 / 16.0)

                    wvw = wvwp.tile([P, W], mm_dt, tag="wvw")
                    wvw_v = wvw[:, :].rearrange("p (h c) -> p h c", h=H)
                    nc.scalar.activation(
                        out=wvw_v[:, :, 32], in_=logits[:, :],
                        func=mybir.ActivationFunctionType.Exp, scale=1.0 / 16.0)
                    w_b = wvw_v[:, :, 32].unsqueeze(2).broadcast_to([P, H, 32])
                    nc.vector.tensor_tensor(
                        out=wvw_v[:, :, 0:32],
                        in0=kvt[:, 256:512].rearrange("p (h c) -> p h c", h=H),
                        in1=w_b, op=mybir.AluOpType.mult)

                    nc.tensor.matmul(acc[:, :], oh_en[:, :], wvw[:, :],
                                     start=(c == 0), stop=(c == cs - 1))
                    ch += 1

                den_t = finp.tile([P, H], f32, tag="den")
                nc.vector.tensor_scalar_add(den_t[:, :], acc_v[:, :, 32], 1e-30)
                rec = finp.tile([P, H], f32, tag="rec")
                nc.vector.reciprocal(out=rec[:, :], in_=den_t[:, :])
                out_t = finp.tile([P, 256], f32, tag="outt")
                rec_b = rec[:, :].unsqueeze(2).broadcast_to([P, H, 32])
                nc.vector.tensor_tensor(
                    out=out_t[:, :].rearrange("p (h c) -> p h c", h=H),
                    in0=acc_v[:, :, 0:32], in1=rec_b,
                    op=mybir.AluOpType.mult)
                nc.sync.dma_start(out=outd[s], in_=out_t[:, :])
                nc.sync.dma_start(
                    out=pld[ch - cs: ch].transpose([1, 0, 2]),
                    in_=pl_t[:, :].rearrange("p (c h) -> p c h", h=H))
    return nc


# ----------------------------------------------------------------------------
# Host-side postprocessing
# ----------------------------------------------------------------------------
def _postprocess(plan, results, metas, dst):
    N, E, T = plan["N"], plan["E"], plan["T"]
    core_tiles = plan["core_tiles"]

    out = np.zeros((N, H, 32), dtype=np.float32)
    prelog = np.zeros((E, H), dtype=np.float32)
    for c in range(NCORES):
        r = results[c]
        o = r["out"].reshape(T, P, H, 32)
        for s in range(T):
            tid = core_tiles[c, s]
            if tid < 0:
                continue
            lo = tid * NPT
            hi = min(lo + NPT, N)
            out[lo:hi] = o[s, : hi - lo]
        m = metas[c]
        valid = m["valid"]
        prelog[m["orig"][valid]] = r["pl"].reshape(-1, H)[valid]

    deg = np.bincount(dst.astype(np.int64), minlength=N)
    out[deg == 0] = 0.0

    out_0 = np.ascontiguousarray(out[:, :, 0:8]).reshape(N, 64, 1)
    out_1 = np.ascontiguousarray(out[:, :, 8:32]).reshape(N, 64, 3)
    return out_0, out_1, prelog


def _ensure_ntff_hook():
    """Register the NTFF profile hook that bass_utils expects under axon.
    The agent image's antenv lacks axon_hooks; synthesize the module and
    wire it to trn_agent_boot's ctypes hook.  Also neuter the cloud
    artifact upload (zero-egress container)."""
    import sys
    import types

    import concourse.bass_utils as bu
    bu.upload_artifacts = lambda tmpdir: "local://" + tmpdir

    try:
        from antenv.axon_hooks import get_axon_ntff_profile_hook  # noqa: F401
        return
    except ImportError:
        pass
    import antenv
    mod = types.ModuleType("antenv.axon_hooks")
    _h = [None]
    mod.set_axon_ntff_profile_hook = lambda h: _h.__setitem__(0, h)
    mod.get_axon_ntff_profile_hook = lambda: _h[0]
    sys.modules["antenv.axon_hooks"] = mod
    antenv.axon_hooks = mod
    from trn_agent_boot.trn_boot import _ntff_profile_via_ctypes
    hook = _ntff_profile_via_ctypes("/opt/axon/libaxon_pjrt.so")
    if hook is not None:
        mod.set_axon_ntff_profile_hook(hook)


# ----------------------------------------------------------------------------
# Entry point
# ----------------------------------------------------------------------------
def kernel(value_0, value_1, key_0, key_1, query_0, query_1, dst,
           _use_f32r=False, _trace=False):
    value_0 = np.asarray(value_0, dtype=np.float32)
    value_1 = np.asarray(value_1, dtype=np.float32)
    key_0 = np.asarray(key_0, dtype=np.float32)
    key_1 = np.asarray(key_1, dtype=np.float32)
    query_0 = np.asarray(query_0, dtype=np.float32)
    query_1 = np.asarray(query_1, dtype=np.float32)
    dst = np.asarray(dst)

    N = query_0.shape[0]
    plan = _plan(dst, N)
    ins, metas = _build_core_inputs(plan, key_0, key_1, query_0, query_1,
                                    value_0, value_1)
    nc = _build_program(plan["T"], plan["n_chunks"], plan["cnt"],
                        use_f32r=_use_f32r)
    if _trace:
        _ensure_ntff_hook()
    res = run_bass_kernel_spmd(nc, ins, list(range(NCORES)), trace=_trace)
    out_0, out_1, prelog = _postprocess(plan, res.results, metas, dst)
    kernel._last_exec_time_ns = res.exec_time_ns
    kernel._last_results = res
    return out_0, out_1, prelog
